# revision 4
# baseline (speedup 1.0000x reference)
"""Trainium2 Bass kernel for multi-head attention (B=2, S=2048, D=1024, H=16, HD=64).

Sharding: tensor-parallel over heads. Each of the 8 cores owns 2 heads
(core c -> heads 2c, 2c+1) and computes:
  - q^T, k^T projections for its heads (layout [head_dim*2, seq])
  - v projection in natural [seq, head_dim*2] layout (with a ones column
    appended per head so the attended matmul also produces the softmax
    denominators for free)
  - scores^T = K @ Q^T per (batch, head) in [key, query] layout, exp on
    ScalarE with the 1/sqrt(64) scale fused into the activation
  - attended^T = [V | 1]^T @ exp(scores^T), normalized by the ones-row sums
  - partial output projection out_c = attended_c @ out_w[:, heads_c]^T
Host sums the 8 partial outputs and adds the bias.

All data transposes are done host-side; the device kernel needs none.
"""

import numpy as np
import ml_dtypes

import concourse.bacc as bacc
import concourse.tile as tile
import concourse.mybir as mybir
from concourse.bass_utils import run_bass_kernel_spmd

B, S, D = 2, 2048, 1024
H, HD = 16, 64
FEA = H * HD  # 1024
NCORES = 8
BS = B * S  # 4096

DT_TILES = 8      # 1024 contraction dim / 128
JT = 16           # key tiles of 128 per batch
IB = 4            # query blocks of 512 per batch
VW = 130          # v storage width per s-tile: [vA(64) | 1 | vB(64) | 1]

BF16 = mybir.dt.bfloat16
F32 = mybir.dt.float32
AF = mybir.ActivationFunctionType
ALU = mybir.AluOpType

_NC_CACHE = {}


def _emit(tc, xT, wqk, wv, wo, out):
    nc = tc.nc
    with (
        tc.tile_pool(name="consts", bufs=1) as consts,
        tc.tile_pool(name="stp", bufs=2) as stp,
        tc.tile_pool(name="small", bufs=4) as small,
        tc.tile_pool(name="tmpb", bufs=2) as tmpb,
        tc.tile_pool(name="outsb", bufs=3) as outsb,
        tc.tile_pool(name="ps_big", bufs=2, space="PSUM") as ps_big,
        tc.tile_pool(name="ps_small", bufs=2, space="PSUM") as ps_small,
    ):
        xts = [consts.tile([128, BS], BF16, name=f"xt{i}", tag=f"xt{i}") for i in range(DT_TILES)]
        wqks = [consts.tile([128, 256], BF16, name=f"wqk{i}", tag=f"wqk{i}") for i in range(DT_TILES)]
        wvs = [consts.tile([128, 128], BF16, name=f"wv{i}", tag=f"wv{i}") for i in range(DT_TILES)]
        wos = consts.tile([128, D], BF16, tag="wo")
        # per-batch activations
        qTs = [consts.tile([128, S], BF16, name=f"qT{b}", tag=f"qT{b}") for b in range(B)]
        kTs = [consts.tile([128, S], BF16, name=f"kT{b}", tag=f"kT{b}") for b in range(B)]
        vsbs = [consts.tile([128, 16 * VW], BF16, name=f"v{b}", tag=f"v{b}") for b in range(B)]
        attTs = [consts.tile([128, S], BF16, name=f"attT{b}", tag=f"attT{b}") for b in range(B)]

        for i in range(DT_TILES):
            nc.sync.dma_start(out=xts[i], in_=xT[i * 128:(i + 1) * 128, :])
            nc.sync.dma_start(out=wqks[i], in_=wqk[i * 128:(i + 1) * 128, :])
            nc.sync.dma_start(out=wvs[i], in_=wv[i * 128:(i + 1) * 128, :])
        nc.sync.dma_start(out=wos, in_=wo[:, :])
        for b in range(B):
            nc.vector.memset(vsbs[b], 1.0)  # presets the ones columns

        def emit_qkv(b):
            # q^T and k^T: [128 fdim, 512] blocks over this batch's seq cols
            for nb in range(4):
                scol = nb * 512
                for half, dst in ((0, qTs[b]), (1, kTs[b])):
                    ps = ps_small.tile([128, 512], F32, name="pss", tag="ps_small")
                    for dt in range(DT_TILES):
                        nc.tensor.matmul(
                            ps,
                            lhsT=wqks[dt][:, half * 128:(half + 1) * 128],
                            rhs=xts[dt][:, b * S + scol: b * S + scol + 512],
                            start=(dt == 0),
                            stop=(dt == DT_TILES - 1),
                        )
                    nc.vector.tensor_copy(out=dst[:, scol:scol + 512], in_=ps)
            # v in natural [s, 64*2] layout, strided into [vA | 1 | vB | 1]
            for st in range(16):
                ps = ps_small.tile([128, 512], F32, name="pss", tag="ps_small")
                for dt in range(DT_TILES):
                    nc.tensor.matmul(
                        ps[:, 0:128],
                        lhsT=xts[dt][:, (b * 16 + st) * 128: (b * 16 + st + 1) * 128],
                        rhs=wvs[dt],
                        start=(dt == 0),
                        stop=(dt == DT_TILES - 1),
                    )
                src = ps[:, 0:128].rearrange("p (two c) -> p two c", two=2)
                dst = vsbs[b][:, st * VW: st * VW + VW].rearrange(
                    "p (two c) -> p two c", two=2
                )[:, :, 0:64]
                nc.vector.tensor_copy(out=dst, in_=src)

        def emit_attention(b):
            for ib in range(IB):
                icol = ib * 512
                for h in range(2):
                    hsl = slice(h * 64, (h + 1) * 64)
                    st_exp = stp.tile([128, JT * 512], BF16, name="st", tag="st")
                    # scores^T -> exp, in groups of 3 key-tiles (3 PSUM banks)
                    for g0 in range(0, JT, 3):
                        gw = min(3, JT - g0)
                        sc = ps_big.tile([128, 1536], F32, name="sc", tag="sc")
                        for idx in range(gw):
                            jt = g0 + idx
                            nc.tensor.matmul(
                                sc[:, idx * 512:(idx + 1) * 512],
                                lhsT=kTs[b][hsl, jt * 128:(jt + 1) * 128],
                                rhs=qTs[b][hsl, icol:icol + 512],
                                start=True,
                                stop=True,
                            )
                        nc.scalar.activation(
                            out=st_exp[:, g0 * 512:(g0 + gw) * 512],
                            in_=sc[:, 0:gw * 512],
                            func=AF.Exp,
                            scale=0.125,
                        )
                    # attended^T (rows 0:64) + softmax denominators (row 64).
                    # Emitted after all score groups so the PSUM slot is held
                    # only ~4us (16 back-to-back matmuls + drain).
                    att_ps = ps_small.tile([128, 512], F32, name="pss", tag="ps_small")
                    for jt in range(JT):
                        nc.tensor.matmul(
                            att_ps[0:65, :],
                            lhsT=vsbs[b][:, jt * VW + h * 65: jt * VW + h * 65 + 65],
                            rhs=st_exp[:, jt * 512:(jt + 1) * 512],
                            start=(jt == 0),
                            stop=(jt == JT - 1),
                        )
                    recip = small.tile([1, 512], F32, name="recip", tag="recip")
                    nc.vector.reciprocal(recip, att_ps[64:65, :])
                    rb = small.tile([64, 512], F32, name="rb", tag="rb")
                    nc.gpsimd.partition_broadcast(rb, recip)
                    if h == 0:
                        nc.vector.tensor_tensor(
                            out=attTs[b][0:64, icol:icol + 512],
                            in0=att_ps[0:64, :],
                            in1=rb,
                            op=ALU.mult,
                        )
                    else:
                        tb = tmpb.tile([64, 512], BF16, name="tb", tag="tb")
                        nc.vector.tensor_tensor(
                            out=tb,
                            in0=att_ps[0:64, :],
                            in1=rb,
                            op=ALU.mult,
                        )
                        # head B lives at partitions 64:128 of attT; engines
                        # can't cross partitions, DMA can.
                        nc.sync.dma_start(
                            out=attTs[b][64:128, icol:icol + 512], in_=tb
                        )

        def emit_outproj(b):
            for st in range(16):
                for db in range(2):
                    ps = ps_small.tile([128, 512], F32, name="pss", tag="ps_small")
                    nc.tensor.matmul(
                        ps,
                        lhsT=attTs[b][:, st * 128:(st + 1) * 128],
                        rhs=wos[:, db * 512:(db + 1) * 512],
                        start=True,
                        stop=True,
                    )
                    osb = outsb.tile([128, 512], F32, name="osb", tag="osb")
                    nc.vector.tensor_copy(out=osb, in_=ps)
                    nc.sync.dma_start(
                        out=out[(b * 16 + st) * 128:(b * 16 + st + 1) * 128,
                                db * 512:(db + 1) * 512],
                        in_=osb,
                    )

        emit_qkv(0)
        emit_qkv(1)
        emit_attention(0)
        emit_outproj(0)
        emit_attention(1)
        emit_outproj(1)


def build_nc():
    if "nc" in _NC_CACHE:
        return _NC_CACHE["nc"]
    nc = bacc.Bacc("TRN2", debug=False, num_devices=NCORES)
    xT = nc.dram_tensor("xT", [D, BS], BF16, kind="ExternalInput").ap()
    wqk = nc.dram_tensor("wqk", [D, 256], BF16, kind="ExternalInput").ap()
    wv = nc.dram_tensor("wv", [D, 128], BF16, kind="ExternalInput").ap()
    wo = nc.dram_tensor("wo", [128, D], BF16, kind="ExternalInput").ap()
    out = nc.dram_tensor("out", [BS, D], F32, kind="ExternalOutput").ap()
    with tile.TileContext(nc) as tc:
        _emit(tc, xT, wqk, wv, wo, out)
    nc.compile()
    _NC_CACHE["nc"] = nc
    return nc


def make_in_maps(x, qkv_w):
    """Host-side shard + transpose + cast. Returns per-core input dicts
    (without wo/out, added by caller)."""
    bf = ml_dtypes.bfloat16
    xT = np.ascontiguousarray(x.reshape(BS, D).T).astype(bf)
    maps = []
    for c in range(NCORES):
        wA = qkv_w[c * 384: c * 384 + 192]
        wB = qkv_w[c * 384 + 192: c * 384 + 384]
        wq = np.concatenate([wA[0:64], wB[0:64]], 0)        # [128, D]
        wk = np.concatenate([wA[64:128], wB[64:128]], 0)    # [128, D]
        wv_ = np.concatenate([wA[128:192], wB[128:192]], 0)  # [128, D]
        wqk_c = np.ascontiguousarray(
            np.concatenate([wq, wk], 0).T).astype(bf)        # [D, 256]
        wv_c = np.ascontiguousarray(wv_.T).astype(bf)        # [D, 128]
        maps.append({"xT": xT, "wqk": wqk_c, "wv": wv_c})
    return maps


def kernel(x, qkv_w, out_w, out_b, _run_kwargs=None):
    x = np.asarray(x, dtype=np.float32)
    qkv_w = np.asarray(qkv_w, dtype=np.float32)
    out_w = np.asarray(out_w, dtype=np.float32)
    out_b = np.asarray(out_b, dtype=np.float32)
    bf = ml_dtypes.bfloat16

    nc = build_nc()
    in_maps = make_in_maps(x, qkv_w)
    for c in range(NCORES):
        wo_c = np.ascontiguousarray(
            out_w[:, c * 128:(c + 1) * 128].T).astype(bf)    # [128, D]
        in_maps[c]["wo"] = wo_c

    res = run_bass_kernel_spmd(
        nc, in_maps, list(range(NCORES)), **(_run_kwargs or {})
    )
    total = np.zeros((BS, D), np.float32)
    for c in range(NCORES):
        total += np.asarray(res.results[c]["out"])
    total += out_b[None, :]
    out = total.reshape(B, S, D)
    if _run_kwargs:
        kernel.last_result = res
    return out


# revision 13
# speedup vs baseline: 1.0165x; 1.0165x over previous
"""Trainium2 Bass kernel for multi-head attention (B=2, S=2048, D=1024, H=16, HD=64).

Sharding: tensor-parallel over heads. Each of the 8 cores owns 2 heads
(core c -> heads 2c, 2c+1) and computes:
  - q^T, k^T projections for its heads (layout [head_dim*2, seq])
  - v projection in natural [seq, head_dim*2] layout (with a ones column
    appended per head so the attended matmul also produces the softmax
    denominators for free)
  - scores^T = K @ Q^T per (batch, head) in [key, query] layout, exp on
    ScalarE with the 1/sqrt(64) scale fused into the activation
  - attended^T = [V | 1]^T @ exp(scores^T), normalized by the ones-row sums
  - partial output projection out_c = attended_c @ out_w[:, heads_c]^T
Host sums the 8 partial outputs and adds the bias.

All data transposes are done host-side; the device kernel needs none.
"""

import numpy as np
import ml_dtypes

import concourse.bacc as bacc
import concourse.tile as tile
import concourse.mybir as mybir
from concourse.bass_utils import run_bass_kernel_spmd

B, S, D = 2, 2048, 1024
H, HD = 16, 64
FEA = H * HD  # 1024
NCORES = 8
BS = B * S  # 4096

DT_TILES = 8      # 1024 contraction dim / 128
JT = 16           # key tiles of 128 per batch
IB = 4            # query blocks of 512 per batch
VW = 256          # v storage width per s-tile: [1|pad63|vA(64) | 1|pad63|vB(64)]

BF16 = mybir.dt.bfloat16
F32 = mybir.dt.float32
AF = mybir.ActivationFunctionType
ALU = mybir.AluOpType

_NC_CACHE = {}


def _emit(tc, xT, wqk, wv, wo, out):
    nc = tc.nc
    with (
        tc.tile_pool(name="consts", bufs=1) as consts,
        tc.tile_pool(name="stp", bufs=2) as stp,
        tc.tile_pool(name="small", bufs=4) as small,
        tc.tile_pool(name="tmpb", bufs=2) as tmpb,
        tc.tile_pool(name="outsb", bufs=3) as outsb,
        tc.tile_pool(name="ps_big", bufs=2, space="PSUM") as ps_big,
        tc.tile_pool(name="ps_small", bufs=2, space="PSUM") as ps_small,
    ):
        xts = [consts.tile([128, BS], BF16, name=f"xt{i}", tag=f"xt{i}") for i in range(DT_TILES)]
        wqks = [consts.tile([128, 256], BF16, name=f"wqk{i}", tag=f"wqk{i}") for i in range(DT_TILES)]
        wvs = [consts.tile([128, 128], BF16, name=f"wv{i}", tag=f"wv{i}") for i in range(DT_TILES)]
        wos = consts.tile([128, D], BF16, tag="wo")
        # per-batch activations
        qTs = [consts.tile([128, S], BF16, name=f"qT{b}", tag=f"qT{b}") for b in range(B)]
        kTs = [consts.tile([128, S], BF16, name=f"kT{b}", tag=f"kT{b}") for b in range(B)]
        vsbs = [consts.tile([128, 16 * VW], BF16, name=f"v{b}", tag=f"v{b}") for b in range(B)]
        attTs = [consts.tile([128, S], BF16, name=f"attT{b}", tag=f"attT{b}") for b in range(B)]

        for i in range(DT_TILES):
            nc.sync.dma_start(out=wqks[i], in_=wqk[i * 128:(i + 1) * 128, :])
            nc.sync.dma_start(out=wvs[i], in_=wv[i * 128:(i + 1) * 128, :])
        nc.sync.dma_start(out=wos, in_=wo[:, :])
        for i in range(DT_TILES):
            nc.sync.dma_start(out=xts[i], in_=xT[i * 128:(i + 1) * 128, :])
        for b in range(B):
            nc.vector.memset(vsbs[b], 1.0)  # presets the ones columns

        def emit_qkv(b):
            # q^T and k^T: [128 fdim, 512] blocks over this batch's seq cols
            for nb in range(4):
                scol = nb * 512
                for half, dst in ((0, qTs[b]), (1, kTs[b])):
                    ps = ps_small.tile([128, 512], F32, name="pss", tag="ps_small")
                    for dt in range(DT_TILES):
                        nc.tensor.matmul(
                            ps,
                            lhsT=wqks[dt][:, half * 128:(half + 1) * 128],
                            rhs=xts[dt][:, b * S + scol: b * S + scol + 512],
                            start=(dt == 0),
                            stop=(dt == DT_TILES - 1),
                        )
                    nc.vector.tensor_copy(out=dst[:, scol:scol + 512], in_=ps)
            # v in natural [s, 64*2] layout, strided into [vA | 1 | vB | 1]
            for st in range(16):
                ps = ps_small.tile([128, 512], F32, name="pss", tag="ps_small")
                for dt in range(DT_TILES):
                    nc.tensor.matmul(
                        ps[:, 0:128],
                        lhsT=xts[dt][:, (b * 16 + st) * 128: (b * 16 + st + 1) * 128],
                        rhs=wvs[dt],
                        start=(dt == 0),
                        stop=(dt == DT_TILES - 1),
                    )
                src = ps[:, 0:128].rearrange("p (two c) -> p two c", two=2)
                # per s-tile: [1|pad63|vA(64) | 1|pad63|vB(64)]. Ones column
                # first => softmax denominators land on PSUM partition 0
                # (partition_broadcast needs a partition-0 source); attended
                # rows occupy partitions 64:128 (spans >32 partitions must
                # start at 0 or 64). Pad columns are 1.0 -> harmless
                # duplicate denominator rows in PSUM.
                dst = vsbs[b][:, st * VW: st * VW + VW].rearrange(
                    "p (two c) -> p two c", two=2
                )[:, :, 64:128]
                nc.vector.tensor_copy(out=dst, in_=src)

        def emit_outproj_ib(b, ib):
            # output rows [b*S + ib*512, +512) only need attT cols of this ib
            for st in range(b * 16 + ib * 4, b * 16 + ib * 4 + 4):
                for db in range(2):
                    ps = ps_small.tile([128, 512], F32, name="pss", tag="ps_small")
                    nc.tensor.matmul(
                        ps,
                        lhsT=attTs[b][:, (st - b * 16) * 128:(st - b * 16 + 1) * 128],
                        rhs=wos[:, db * 512:(db + 1) * 512],
                        start=True,
                        stop=True,
                    )
                    osb = outsb.tile([128, 512], F32, name="osb", tag="osb")
                    nc.vector.tensor_copy(out=osb, in_=ps)
                    nc.sync.dma_start(
                        out=out[st * 128:(st + 1) * 128, db * 512:(db + 1) * 512],
                        in_=osb,
                    )

        def emit_attention(b):
            for ib in range(IB):
                icol = ib * 512
                for h in range(2):
                    hsl = slice(h * 64, (h + 1) * 64)
                    st_exp = stp.tile([128, JT * 512], BF16, name="st", tag="st")
                    # scores^T -> exp, in groups of 3 key-tiles (3 PSUM banks)
                    for g0 in range(0, JT, 3):
                        gw = min(3, JT - g0)
                        sc = ps_big.tile([128, 1536], F32, name="sc", tag="sc")
                        for idx in range(gw):
                            jt = g0 + idx
                            nc.tensor.matmul(
                                sc[:, idx * 512:(idx + 1) * 512],
                                lhsT=kTs[b][hsl, jt * 128:(jt + 1) * 128],
                                rhs=qTs[b][hsl, icol:icol + 512],
                                start=True,
                                stop=True,
                            )
                        nc.scalar.activation(
                            out=st_exp[:, g0 * 512:(g0 + gw) * 512],
                            in_=sc[:, 0:gw * 512],
                            func=AF.Exp,
                            scale=0.125,
                        )
                    # attended^T (rows 0:64) + softmax denominators (row 64).
                    # Emitted after all score groups so the PSUM slot is held
                    # only ~4us (16 back-to-back matmuls + drain).
                    att_ps = ps_small.tile([128, 512], F32, name="pss", tag="ps_small")
                    for jt in range(JT):
                        nc.tensor.matmul(
                            att_ps[0:128, :],
                            lhsT=vsbs[b][:, jt * VW + h * 128: jt * VW + (h + 1) * 128],
                            rhs=st_exp[:, jt * 512:(jt + 1) * 512],
                            start=(jt == 0),
                            stop=(jt == JT - 1),
                        )
                    # Drain PSUM immediately (one copy) so the slot frees
                    # fast; normalize from the SBUF copy. Row 0 =
                    # denominators, rows 64:128 = attended^T.
                    araw = small.tile([128, 512], F32, name="araw", tag="araw")
                    nc.vector.tensor_copy(out=araw, in_=att_ps[0:128, :])
                    rrow = small.tile([1, 512], F32, name="rrow", tag="rrow")
                    nc.vector.reciprocal_approx_fast(out=rrow, in_=araw[0:1, :])
                    rb = small.tile([128, 512], F32, name="rb", tag="rb")
                    nc.gpsimd.partition_broadcast(rb, rrow)
                    # tensor_tensor needs both SBUF inputs at the same base
                    # partition -> use the 64:128 half of the broadcast.
                    if h == 0:
                        nc.vector.tensor_tensor(
                            out=attTs[b][0:64, icol:icol + 512],
                            in0=araw[64:128, :],
                            in1=rb[64:128, :],
                            op=ALU.mult,
                        )
                    else:
                        tb = tmpb.tile([64, 512], BF16, name="tb", tag="tb")
                        nc.vector.tensor_tensor(
                            out=tb,
                            in0=araw[64:128, :],
                            in1=rb[64:128, :],
                            op=ALU.mult,
                        )
                        # head B lives at partitions 64:128 of attT; engines
                        # can't cross partitions, DMA can.
                        nc.sync.dma_start(
                            out=attTs[b][64:128, icol:icol + 512], in_=tb
                        )
                emit_outproj_ib(b, ib)

        emit_qkv(0)
        emit_attention(0)
        emit_qkv(1)
        emit_attention(1)


def build_nc():
    if "nc" in _NC_CACHE:
        return _NC_CACHE["nc"]
    nc = bacc.Bacc("TRN2", debug=False, num_devices=NCORES)
    xT = nc.dram_tensor("xT", [D, BS], BF16, kind="ExternalInput").ap()
    wqk = nc.dram_tensor("wqk", [D, 256], BF16, kind="ExternalInput").ap()
    wv = nc.dram_tensor("wv", [D, 128], BF16, kind="ExternalInput").ap()
    wo = nc.dram_tensor("wo", [128, D], BF16, kind="ExternalInput").ap()
    out = nc.dram_tensor("out", [BS, D], F32, kind="ExternalOutput").ap()
    with tile.TileContext(nc) as tc:
        _emit(tc, xT, wqk, wv, wo, out)
    nc.compile()
    _NC_CACHE["nc"] = nc
    return nc


def make_in_maps(x, qkv_w):
    """Host-side shard + transpose + cast. Returns per-core input dicts
    (without wo/out, added by caller)."""
    bf = ml_dtypes.bfloat16
    xT = np.ascontiguousarray(x.reshape(BS, D).T).astype(bf)
    maps = []
    for c in range(NCORES):
        wA = qkv_w[c * 384: c * 384 + 192]
        wB = qkv_w[c * 384 + 192: c * 384 + 384]
        wq = np.concatenate([wA[0:64], wB[0:64]], 0)        # [128, D]
        wk = np.concatenate([wA[64:128], wB[64:128]], 0)    # [128, D]
        wv_ = np.concatenate([wA[128:192], wB[128:192]], 0)  # [128, D]
        wqk_c = np.ascontiguousarray(
            np.concatenate([wq, wk], 0).T).astype(bf)        # [D, 256]
        wv_c = np.ascontiguousarray(wv_.T).astype(bf)        # [D, 128]
        maps.append({"xT": xT, "wqk": wqk_c, "wv": wv_c})
    return maps


def kernel(x, qkv_w, out_w, out_b, _run_kwargs=None):
    x = np.asarray(x, dtype=np.float32)
    qkv_w = np.asarray(qkv_w, dtype=np.float32)
    out_w = np.asarray(out_w, dtype=np.float32)
    out_b = np.asarray(out_b, dtype=np.float32)
    bf = ml_dtypes.bfloat16

    nc = build_nc()
    in_maps = make_in_maps(x, qkv_w)
    for c in range(NCORES):
        wo_c = np.ascontiguousarray(
            out_w[:, c * 128:(c + 1) * 128].T).astype(bf)    # [128, D]
        in_maps[c]["wo"] = wo_c

    res = run_bass_kernel_spmd(
        nc, in_maps, list(range(NCORES)), **(_run_kwargs or {})
    )
    total = np.zeros((BS, D), np.float32)
    for c in range(NCORES):
        total += np.asarray(res.results[c]["out"])
    total += out_b[None, :]
    out = total.reshape(B, S, D)
    if _run_kwargs:
        kernel.last_result = res
    return out
